# revision 47
# baseline (speedup 1.0000x reference)
"""Trainium2 Bass kernel for nn_AttentionBlock (sparse attention block).

Math (per batch b, position t):
  att = concat([q, k, q-k, q*k]) @ W1  ==  q@(W1a+W1c) + k@(W1b-W1c) + (q*k)@W1d
  h1 = relu(att + b1); h2 = relu(h1@W2 + b2); s = h2@W3 + b3
  s = where(behavior==0, s, PAD)/sqrt(D); w = softmax(s); out = w @ keys

Only positions with behavior==0 (~20%, max 63 of 200 with this data
distribution) survive the mask, so we gather just those key rows per batch
(G=64 slots) with indirect DMA and run the whole MLP + attention on the
gathered slots. b3 is dropped (softmax shift invariance). 1/sqrt(D) is folded
into W3. Data-parallel over batch across 8 cores.

Device layout (per core, Bs=256 batches, halves of 128):
  - pair j = (batch j, batch j+64) within a half; gathers are batched 16
    pairs (= 4 gl iterations) per indirect DMA -> [128, 2048] tile, which
    amortizes the 994ns SWDGE fixed overhead 16x.
  - PE-transposes pairs into kT [128 D, 512] using a bf16 identity (1
    cycle/row) on f32r data; kT copied PSUM->SBUF on the Pool engine.
  - L1 = sel-term (host-precomputed u = q@Wa+b1, expanded over slot columns
    by a constant one-hot selector matmul) + wB@kT + wD@(kT*q) accumulated
    in PSUM; the q broadcast for the elementwise multiply is a stride-0 AP
    (nothing materialized).
  - L2 fp32r; L3 (M=1) lands 4 iterations into one PSUM tile at partition
    {0,32,64,96} via tile_position, then scores are scatter-DMA'd directly
    from PSUM into [128 batch, 64 slot] tiles (no SBUF staging).
  - masked softmax as before; the weight matrix wext is PE-transposed to
    wT [slot, batch] once per half.
  - final attention inverted: per pair ONE matmul with the gathered keys
    kg [slot, D] as the f32r stationary operand and wT columns {j, j+64}
    as the N=2 moving operand, accumulating out.T [D, batch-cols] in one
    PSUM tile per half; single copy + single DMA per half writes outT
    [D, Bs] (host transposes back).
"""

import sys

import numpy as np

sys.path.insert(0, "/opt/trn_rl_repo")

import concourse.bacc as bacc  # noqa: E402
import concourse.tile as tile  # noqa: E402
from concourse import mybir  # noqa: E402
from concourse.bass import IndirectOffsetOnAxis  # noqa: E402
from concourse.masks import make_identity  # noqa: E402

F32 = mybir.dt.float32
F32R = mybir.dt.float32r
BF16 = mybir.dt.bfloat16
I32 = mybir.dt.int32

B, T, D = 2048, 200, 128
G = 64  # gathered slots per batch
P = 128
NCORES = 8
PAD_NEG = -1.0e9
G4 = 4  # gl iterations per batched gather


def r32(ap):
    return ap.bitcast(F32R)


def build_nc(Bs):
    """Build the per-core Bass program. Bs = batches per core (multiple of 128)."""
    halves = Bs // P
    npair = Bs // 2  # pairs of batches, (j, j+64) within each half

    nc = bacc.Bacc(None)
    keys = nc.declare_dram_parameter("keysflat", [Bs * T, D], F32, isOutput=False)
    gidx = nc.declare_dram_parameter("gidx", [P, npair], I32, isOutput=False)
    qTp = nc.declare_dram_parameter("qTp", [D, Bs], F32, isOutput=False)
    uT2 = nc.declare_dram_parameter("uT2", [8, halves * 16 * 80], F32, isOutput=False)
    sel = nc.declare_dram_parameter("sel", [8, 512], F32, isOutput=False)
    cnt = nc.declare_dram_parameter("counts", [P, halves], F32, isOutput=False)
    wB = nc.declare_dram_parameter("wB", [D, 80], F32, isOutput=False)
    wD = nc.declare_dram_parameter("wD", [D, 80], F32, isOutput=False)
    w2 = nc.declare_dram_parameter("w2", [80, 128], F32, isOutput=False)
    w3 = nc.declare_dram_parameter("w3", [128, 32], F32, isOutput=False)
    b2 = nc.declare_dram_parameter("b2", [128, 1], F32, isOutput=False)
    outT = nc.declare_dram_parameter("outT", [D, Bs], F32, isOutput=True)

    from contextlib import ExitStack

    with tile.TileContext(nc) as tc:
        with ExitStack() as ctx:
            pool = lambda *a, **k: ctx.enter_context(tc.tile_pool(*a, **k))  # noqa: E731
            const = pool(name="const", bufs=1)
            kgp = pool(name="kg", bufs=132)
            ktp = pool(name="kt", bufs=3)
            qkp = pool(name="qk", bufs=3)
            h1p = pool(name="h1", bufs=2)
            h2p = pool(name="h2", bufs=2)
            scp = pool(name="scores", bufs=2)
            smp = pool(name="sm", bufs=2)
            padp = pool(name="pad", bufs=2)
            smallp = pool(name="small", bufs=8)
            sstp = pool(name="sst", bufs=2)
            wep = pool(name="wext", bufs=2)
            wtp = pool(name="wt", bufs=2)
            otp = pool(name="ot", bufs=2)
            psK = pool(name="psK", bufs=2, space="PSUM")
            psH1 = pool(name="psH1", bufs=2, space="PSUM")
            psH2 = pool(name="psH2", bufs=1, space="PSUM")
            psS = pool(name="psS", bufs=2, space="PSUM")
            psWO = pool(name="psWO", bufs=1, space="PSUM")

            # ---- constants / inputs staged once ----
            ident = const.tile([P, P], F32)
            make_identity(nc, ident[:])
            gidx_sb = const.tile([P, npair], I32)
            nc.sync.dma_start(out=gidx_sb[:], in_=gidx[:])
            qTp_sb = const.tile([D, Bs], F32)
            nc.sync.dma_start(out=qTp_sb[:], in_=qTp[:])
            uT2_s0 = const.tile([8, halves * 16 * 80], F32)
            nc.sync.dma_start(out=uT2_s0[:], in_=uT2[:])
            sel_s0 = const.tile([8, 512], F32)
            nc.sync.dma_start(out=sel_s0[:], in_=sel[:])
            cnt_sb = const.tile([P, halves], F32)
            nc.sync.dma_start(out=cnt_sb[:], in_=cnt[:])
            wB_s0 = const.tile([D, 80], F32)
            nc.sync.dma_start(out=wB_s0[:], in_=wB[:])
            wD_s0 = const.tile([D, 80], F32)
            nc.sync.dma_start(out=wD_s0[:], in_=wD[:])
            w2_s0 = const.tile([80, 128], F32)
            nc.sync.dma_start(out=w2_s0[:], in_=w2[:])
            w3_s0 = const.tile([128, 32], F32)
            nc.sync.dma_start(out=w3_s0[:], in_=w3[:])
            b2_sb = const.tile([128, 1], F32)
            nc.sync.dma_start(out=b2_sb[:], in_=b2[:])
            # f32r matmul operands must be engine-rounded, not DMA-written
            uT2_sb = const.tile([8, halves * 16 * 80], F32R)
            nc.vector.tensor_copy(uT2_sb[:], uT2_s0[:])
            sel_sb = const.tile([8, 512], F32R)
            nc.vector.tensor_copy(sel_sb[:], sel_s0[:])
            wB_sb = const.tile([D, 80], F32R)
            nc.vector.tensor_copy(wB_sb[:], wB_s0[:])
            wD_sb = const.tile([D, 80], F32R)
            nc.vector.tensor_copy(wD_sb[:], wD_s0[:])
            w2_sb = const.tile([80, 128], F32R)
            nc.vector.tensor_copy(w2_sb[:], w2_s0[:])
            w3_sb = const.tile([128, 32], BF16)
            nc.vector.tensor_copy(w3_sb[:], w3_s0[:])

            iota_f = const.tile([P, G], F32)
            iota_i = const.tile([P, G], I32)
            nc.gpsimd.iota(iota_i[:], pattern=[[1, G]], base=0, channel_multiplier=0)
            nc.vector.tensor_copy(iota_f[:], iota_i[:])
            pad_t = const.tile([P, G], F32)
            nc.vector.memset(pad_t[:], PAD_NEG)

            # batch-in-core of (h, gl, p, eo) = 128h + 64eo + 4gl + p
            for h in range(halves):
                kg_tiles = []
                sst = sstp.tile([P, 2048], F32)
                wext = wep.tile([P, P], F32)
                nc.vector.memset(wext[:], 0.0)
                ot = otp.tile([P, P], F32)
                # ---- phase A: per-pair gather + MLP -> scores ----
                # (one [128,1]-offset indirect DMA per pair: the only
                # indirect-DMA form with validated hw semantics)
                for g4 in range(16 // G4):
                    for t0 in range(G4):
                        for p0 in range(4):
                            c = h * 64 + 4 * (G4 * g4 + t0) + p0
                            kgt = kgp.tile([P, 128], F32)
                            nc.gpsimd.indirect_dma_start(
                                out=kgt[:],
                                out_offset=None,
                                in_=keys[:],
                                in_offset=IndirectOffsetOnAxis(
                                    ap=gidx_sb[:, c : c + 1], axis=0
                                ),
                            )
                            kg_tiles.append(kgt)
                    for t_ in range(G4):
                        gl = G4 * g4 + t_
                        ps_kT = psK.tile([P, 512], F32)
                        for p_ in range(4):
                            nc.tensor.transpose(
                                out=ps_kT[:, 128 * p_ : 128 * p_ + 128],
                                in_=kg_tiles[4 * gl + p_][:],
                                identity=ident[:],
                            )
                        kT = ktp.tile([P, 512], F32R)
                        nc.vector.tensor_copy(kT[:], ps_kT[:])
                        # per-batch q columns broadcast over the 64 slots;
                        # kT/qk column order: (p, eo, s) -> batch
                        # 128h + 64eo + 4gl + p, qTp col order (h, gl, eo, p)
                        qbc = (
                            r32(qTp_sb[:, 8 * (16 * h + gl) : 8 * (16 * h + gl) + 8])
                            .rearrange("d (eo p) -> d p eo", eo=2, p=4)
                            .to_broadcast([D, 4, 2, G])
                        )
                        qk = qkp.tile([P, 512], F32R)
                        nc.vector.tensor_tensor(
                            out=qk[:].rearrange("d (p eo s) -> d p eo s", p=4, eo=2),
                            in0=kT[:].rearrange("d (p eo s) -> d p eo s", p=4, eo=2),
                            in1=qbc,
                            op=mybir.AluOpType.mult,
                        )
                        ps_h1 = psH1.tile([80, 512], F32)
                        u0 = 80 * (16 * h + gl)
                        nc.tensor.matmul(
                            ps_h1[:],
                            uT2_sb[:, u0 : u0 + 80],
                            sel_sb[:],
                            start=True,
                            stop=False,
                            tile_position=(0, 0),
                        )
                        nc.tensor.matmul(
                            ps_h1[:], wB_sb[:], kT[:], start=False, stop=False
                        )
                        nc.tensor.matmul(
                            ps_h1[:], wD_sb[:], qk[:], start=False, stop=True
                        )
                        h1 = h1p.tile([80, 512], F32R)
                        nc.scalar.activation(
                            h1[:],
                            ps_h1[:],
                            mybir.ActivationFunctionType.Relu,
                            bias=0.0,
                            scale=1.0,
                        )
                        ps_h2 = psH2.tile([P, 512], F32)
                        nc.tensor.matmul(
                            ps_h2[:], w2_sb[:], h1[:], start=True, stop=True
                        )
                        h2 = h2p.tile([P, 512], BF16)
                        nc.scalar.activation(
                            h2[:],
                            ps_h2[:],
                            mybir.ActivationFunctionType.Relu,
                            bias=b2_sb[:, 0:1],
                            scale=1.0,
                        )
                        if t_ == 0:
                            ps_s4 = psS.tile([P, 512], F32)
                        nc.tensor.matmul(
                            ps_s4[32 * t_ : 32 * t_ + 32, :],
                            w3_sb[:],
                            h2[:],
                            start=True,
                            stop=True,
                            tile_position=(0, 32 * t_),
                        )
                    # stage scores in SBUF: row 32t, col 1024eo+256g4+64p+s
                    # (one contiguous 256-col block per eo)
                    for eo in range(2):
                        c0b = 1024 * eo + 256 * g4
                        nc.vector.tensor_copy(
                            sst[:, c0b : c0b + 256].rearrange(
                                "q (p s) -> q p s", p=4
                            ),
                            ps_s4[:].rearrange(
                                "q (p eo s) -> q eo p s", p=4, eo=2
                            )[:, eo],
                        )
                    if g4 % 2 == 0:
                        continue
                    # ---- super-group sg = g4//2: scatter + softmax +
                    # phase C for pairs j in [32sg, 32sg+32)  (pair j =
                    # 32sg + 8t + 4(g4%2) + p) while later gathers stream
                    sg = g4 // 2
                    sct = {}
                    for eo in range(2):
                        s_t = scp.tile([P, G], F32, tag=f"sc{sg}{eo}")
                        sct[eo] = s_t
                        nc.vector.memset(s_t[:], 0.0)
                        c0s = 1024 * eo + 512 * sg
                        in_ap = sst[:, c0s : c0s + 512].rearrange(
                            "(k r) (gp s) -> r k gp s", k=G4, r=32, gp=8
                        )[0]
                        r0 = 64 * eo + 32 * sg
                        nc.sync.dma_start(out=s_t[r0 : r0 + 32, :], in_=in_ap)
                    nmax = smallp.tile([P, 1], F32)
                    expt = smp.tile([P, G], F32)
                    sume = smallp.tile([P, 1], F32)
                    rsum = smallp.tile([P, 1], F32)
                    for eo in range(2):
                        r = slice(64 * eo + 32 * sg, 64 * eo + 32 * sg + 32)
                        padm = padp.tile([P, G], I32, tag=f"padm{eo}")
                        nc.vector.tensor_tensor(
                            out=padm[r, :],
                            in0=iota_f[r, :],
                            in1=cnt_sb[r, h : h + 1].to_broadcast([32, G]),
                            op=mybir.AluOpType.is_ge,
                        )
                        nc.vector.copy_predicated(
                            out=sct[eo][r, :], mask=padm[r, :], data=pad_t[r, :]
                        )
                        nc.vector.tensor_reduce(
                            out=nmax[r, :],
                            in_=sct[eo][r, :],
                            axis=mybir.AxisListType.X,
                            op=mybir.AluOpType.max,
                            negate=True,
                        )
                        nc.scalar.activation(
                            expt[r, :],
                            sct[eo][r, :],
                            mybir.ActivationFunctionType.Exp,
                            bias=nmax[r, 0:1],
                            scale=1.0,
                            accum_out=sume[r, 0:1],
                        )
                        nc.vector.reciprocal(rsum[r, :], sume[r, :])
                        # wext[batch, slot-col]: lo -> cols 0:G, hi -> G:2G
                        nc.vector.tensor_tensor(
                            out=wext[r, 64 * eo : 64 * eo + G],
                            in0=expt[r, :],
                            in1=rsum[r, 0:1].to_broadcast([32, G]),
                            op=mybir.AluOpType.mult,
                        )
                    ps_wT = psWO.tile([P, P], F32, tag="wo")
                    nc.tensor.transpose(
                        out=ps_wT[:], in_=wext[:], identity=ident[:]
                    )
                    wT_sb = wtp.tile([P, P], F32)
                    nc.vector.tensor_copy(wT_sb[:], ps_wT[:])
                    ps_o = psWO.tile([P, P], F32, tag="wo")
                    for jl in range(32):
                        j = 32 * sg + jl
                        t_, g4l, p_ = jl // 8, (jl % 8) // 4, jl % 4
                        gl_j = 4 * (2 * sg + g4l) + t_
                        nc.tensor.matmul(
                            ps_o[:, j : j + 65 : 64],
                            kg_tiles[4 * gl_j + p_][:],
                            wT_sb[:, j : j + 65 : 64],
                            start=True,
                            stop=True,
                        )
                    for c0o in (32 * sg, 64 + 32 * sg):
                        nc.scalar.copy(
                            ot[:, c0o : c0o + 32], ps_o[:, c0o : c0o + 32]
                        )
                nc.sync.dma_start(out=outT[:, 128 * h : 128 * h + 128], in_=ot[:])
    nc.compile()
    return nc


def _host_prep(query, keys, behavior_input, W1, b1, W2, b2, W3, b3):
    query = np.ascontiguousarray(np.asarray(query, np.float32).reshape(B, D))
    keys = np.ascontiguousarray(np.asarray(keys, np.float32))
    beh = np.asarray(behavior_input)
    W1 = np.asarray(W1, np.float32)
    Wa = np.ascontiguousarray(W1[0:D] + W1[2 * D : 3 * D])
    Wb = np.ascontiguousarray(W1[D : 2 * D] - W1[2 * D : 3 * D])
    Wd = np.ascontiguousarray(W1[3 * D : 4 * D])
    W3v = np.asarray(W3, np.float32) / np.sqrt(np.float32(D))
    # L2/L3 padded to a full 128-wide hidden so every matmul uses a full
    # 128-row stationary (ISA dst-partition rule) and all PSUM rows are
    # initialized: h2 rows 40:128 = relu(0 + 0) = 0 exactly.
    W3s = np.zeros((128, 32), np.float32)
    W3s[0:40, 0:1] = W3v
    b1c = np.asarray(b1, np.float32).reshape(80, 1)
    b2c = np.asarray(b2, np.float32).reshape(40, 1)

    mask = beh == 0
    counts = mask.sum(1).astype(np.int64)
    order = np.argsort(~mask, axis=1, kind="stable")
    idx = order[:, :G].astype(np.int64)  # [B, G] position indices
    return query, keys, Wa, Wb, Wd, W3s, b1c, b2c, counts, idx


def _pair_of(gl, p):
    """Gather chunk (gl = 4*g4 + t, p) -> pair index j within the half."""
    return 32 * (gl // 8) + 8 * (gl % 4) + 4 * ((gl // 4) % 2) + p


def _perm(Bs):
    """qTp/uT2 column order (h, gl, eo, p) -> local batch 128h+64eo+j."""
    halves = Bs // P
    perm = np.empty(Bs, np.int64)
    c = 0
    for h in range(halves):
        for gl in range(16):
            for eo in range(2):
                for p in range(4):
                    perm[c] = 128 * h + 64 * eo + _pair_of(gl, p)
                    c += 1
    return perm


def _sel():
    s = np.zeros((8, 512), np.float32)
    for p in range(4):
        for eo in range(2):
            s[4 * eo + p, 128 * p + 64 * eo : 128 * p + 64 * eo + 64] = 1.0
    return s


def _numpy_fallback(query, keys, Wa, Wb, Wd, W3s, b1c, b2c, counts, idx, W2f):
    out = np.zeros((B, D), np.float32)
    for b in range(B):
        kg = keys[b, idx[b]]
        q = query[b]
        h1 = np.maximum(kg @ Wb + (q * kg) @ Wd + q @ Wa + b1c[:, 0], 0)
        h2 = np.maximum(h1 @ W2f + b2c[:, 0], 0)
        s = (h2 @ W3s)[:, 0]
        s[counts[b] :] = PAD_NEG
        e = np.exp(s - s.max())
        out[b] = (e / e.sum()) @ kg
    return out


def _gidx_layout(idx, counts, b0, Bs):
    """Device gather-index + counts layout for one core.

    gather col 64h + 4gl + p holds local flat key-row indices for the pair
    j = _pair_of(gl, p): rows 0:64 = batch 128h+j, rows 64:128 = batch
    128h+64+j (local flat row = b_local*T + t).
    """
    halves = Bs // P
    npair = Bs // 2
    gidx_cols = np.zeros((P, npair), np.int32)
    cnt2 = np.zeros((P, halves), np.float32)
    for h in range(halves):
        for gl in range(16):
            for p in range(4):
                c = h * 64 + 4 * gl + p
                blo = 128 * h + _pair_of(gl, p)
                bhi = blo + 64
                gidx_cols[0:64, c] = blo * T + idx[b0 + blo]
                gidx_cols[64:128, c] = bhi * T + idx[b0 + bhi]
        cnt2[:, h] = counts[b0 + 128 * h : b0 + 128 * h + 128]
    return gidx_cols, cnt2


def _core_inputs(query_f, keys_f, Wa, Wb, Wd, W3s, b1c, b2c, counts, idx, W2f, b0, Bs):
    halves = Bs // P
    perm = _perm(Bs)
    qloc = query_f[b0 : b0 + Bs]
    qTp = np.ascontiguousarray(qloc[perm].T)  # [D, Bs]
    u = qloc @ Wa + b1c[:, 0]  # [Bs, 80]
    up = u[perm].reshape(halves, 16, 8, 80)
    # [8, (h, gl) blocks of 80]
    uT2 = np.ascontiguousarray(up.transpose(2, 0, 1, 3).reshape(8, halves * 16 * 80))
    gidx_cols, cnt2 = _gidx_layout(idx, counts, b0, Bs)
    W2p = np.zeros((80, 128), np.float32)
    W2p[:, 0:40] = W2f
    b2p = np.zeros((128, 1), np.float32)
    b2p[0:40] = b2c
    return {
        "keysflat": keys_f[b0 : b0 + Bs].reshape(Bs * T, D),
        "gidx": gidx_cols,
        "qTp": qTp,
        "uT2": uT2,
        "sel": _sel(),
        "counts": cnt2,
        "wB": Wb,
        "wD": Wd,
        "w2": W2p,
        "w3": W3s,
        "b2": b2p,
    }


def kernel(query, keys, behavior_input, W1, b1, W2, b2, W3, b3):
    from concourse.bass_utils import run_bass_kernel_spmd

    (query_f, keys_f, Wa, Wb, Wd, W3s, b1c, b2c, counts, idx) = _host_prep(
        query, keys, behavior_input, W1, b1, W2, b2, W3, b3
    )
    W2f = np.ascontiguousarray(np.asarray(W2, np.float32))
    Bs = B // NCORES

    if counts.max() > G:
        outv = _numpy_fallback(
            query_f, keys_f, Wa, Wb, Wd, W3s, b1c, b2c, counts, idx, W2f
        )
        return _finish(outv, keys_f, counts)

    nc = build_nc(Bs)
    in_maps = [
        _core_inputs(
            query_f, keys_f, Wa, Wb, Wd, W3s, b1c, b2c, counts, idx, W2f, core * Bs, Bs
        )
        for core in range(NCORES)
    ]
    res = run_bass_kernel_spmd(nc, in_maps, core_ids=list(range(NCORES)))
    outv = np.concatenate(
        [np.asarray(res.results[i]["outT"]).T for i in range(NCORES)], axis=0
    )
    return _finish(outv, keys_f, counts)


def _finish(outv, keys_f, counts):
    # rows whose mask selected nothing: reference softmaxes a row of equal PAD
    # values -> uniform average over all T keys
    zrows = np.nonzero(counts == 0)[0]
    for b in zrows:
        outv[b] = keys_f[b].mean(axis=0)
    return outv.reshape(B, 1, D).astype(np.float32)
